# revision 15
# baseline (speedup 1.0000x reference)
"""Trainium2 Bass kernel for an autoregressive LSTM decompressor.

Reference math:
  step 0:    gates = x @ W_ih.T + b            (h = c = 0)
  step t>=1: gates = h_{t-1} @ (W_ih+W_hh).T + b    (input == previous hidden)
  i,f,g,o = split(gates); c = sig(f)*c + sig(i)*tanh(g); h = sig(o)*tanh(c)
  out[b,t,:] = h_{t+1} @ W_out.T + b_out  (rows are h_1..h_T)

Sharding: data-parallel, batch 256 -> 32 per core across 8 cores; combined
weights (W_ih+W_hh) resident in SBUF.

Layout (the whole point of this kernel): per-core batch is only 32, so a
batch-stationary matmul would use 32 of the PE array's 128 columns and
stream 32K weight columns per step. Instead the matmuls run
WEIGHT-STATIONARY in transposed space: stationary = Wc.T block
[K=128 h-rows, M=128 gate-rows] (full array), moving = h.T chunk
[128, 32]. fp8(e4m3) weights make the per-tile LDWEIGHTS use 4x
fast-weight-load, which is what bounds this layout. Gates come out
TRANSPOSED ([gate-rows, batch]), so:
  - gate PSUM banks pack 16 [128,32] blocks: bank A = i|f, bank B = g|o,
    and every elementwise tail op runs at full 128-lane width;
  - the global fp8 dequant scale rides the ACT activation `scale`
    immediate (bias images are pre-divided by it on the host);
  - h.T is produced directly -- NO transposes anywhere in the loop;
  - the output projection is also weight-stationary ([dout, batch] out),
    with per-partition b_out bias riding the ACT bias port; it runs in
    the PE's tail window. Projection uses a bf16 copy of h (fp8 h in the
    linear output projection fails the 2e-2 tolerance; in the recurrence
    the saturating gates forgive it -- measured 1.3e-2 end to end).
Step 0's gates (x @ W_ih.T + b, 1/128 of the FLOPs) are precomputed on
the host so W_ih is never shipped; the output is returned transposed per
core and fixed up on the host.
"""

import os
import numpy as np

B, H, DOUT = 256, 1024, 128
NCORES = 8
BLOC = B // NCORES  # 32
G4 = 4 * H  # 4096

_CACHE = {}
_FLAGS = set()  # experiment flags: no_tail, no_proj, wfp8
_REPS = 1  # timing experiments: repeat the steady-state loop


def _wdt():
    import concourse.mybir as mybir
    return mybir.dt.float8e4 if "wfp8" in _FLAGS else mybir.dt.bfloat16


def _build_program(T):
    import concourse.mybir as mybir
    from concourse import bacc
    from concourse.tile import TileContext

    f32 = mybir.dt.float32
    bf16 = mybir.dt.bfloat16
    w8 = _wdt()
    SIG = mybir.ActivationFunctionType.Sigmoid
    TANH = mybir.ActivationFunctionType.Tanh
    IDENT = mybir.ActivationFunctionType.Identity

    nc = bacc.Bacc("TRN2", target_bir_lowering=False, debug=False,
                   num_devices=NCORES)

    wc_d = nc.dram_tensor("WcT8", [H, G4], w8, kind="ExternalInput").ap()
    wo_d = nc.dram_tensor("WoTb", [H, DOUT], bf16, kind="ExternalInput").ap()
    ba_d = nc.dram_tensor("biasA", [128, 512], f32, kind="ExternalInput").ap()
    bb_d = nc.dram_tensor("biasB", [128, 512], f32, kind="ExternalInput").ap()
    bo_d = nc.dram_tensor("boC", [DOUT, 1], f32, kind="ExternalInput").ap()
    g0a_d = nc.dram_tensor("g0A", [128, 512], f32, kind="ExternalInput").ap()
    g0b_d = nc.dram_tensor("g0B", [128, 512], f32, kind="ExternalInput").ap()
    # transposed output; host rearranges to [BLOC, T, DOUT]
    out_d = nc.dram_tensor("outT", [DOUT, T, BLOC], f32,
                           kind="ExternalOutput").ap()
    SW = float(os.environ.get("KSW", "1.0"))  # fp8 dequant scale immediate
    KDBG = bool(os.environ.get("KDBG"))
    if KDBG:
        gdbg_d = nc.dram_tensor("gdbg", [T, 128, 1024], f32,
                                kind="ExternalOutput").ap()
        hdbg_d = nc.dram_tensor("hdbg", [T, 128, 256], f32,
                                kind="ExternalOutput").ap()

    with TileContext(nc) as tc:
        with (
            tc.tile_pool(name="const", bufs=1) as const_pool,
            tc.tile_pool(name="wc", bufs=1) as wc_pool,
            tc.tile_pool(name="state", bufs=2) as state_pool,
            tc.tile_pool(name="cpool", bufs=1) as c_pool,
            tc.tile_pool(name="ew", bufs=2) as ew_pool,
            tc.tile_pool(name="gates_ps", bufs=2, space="PSUM") as gps_pool,
            tc.tile_pool(name="proj_ps", bufs=2, space="PSUM") as pps_pool,
        ):
            ba_sb = const_pool.tile([128, 512], f32, name="ba_sb")
            nc.sync.dma_start(ba_sb, ba_d)
            bb_sb = const_pool.tile([128, 512], f32, name="bb_sb")
            nc.sync.dma_start(bb_sb, bb_d)
            bo_sb = const_pool.tile([DOUT, 1], f32, name="bo_sb")
            nc.sync.dma_start(bo_sb, bo_d)
            g0a_sb = const_pool.tile([128, 512], f32, name="g0a_sb")
            nc.sync.dma_start(g0a_sb, g0a_d)
            g0b_sb = const_pool.tile([128, 512], f32, name="g0b_sb")
            nc.sync.dma_start(g0b_sb, g0b_d)
            wo_sb = const_pool.tile([128, DOUT * 8], bf16, name="wo_sb")
            for k in range(8):
                nc.sync.dma_start(wo_sb[:, DOUT * k:DOUT * (k + 1)],
                                  wo_d[128 * k:128 * k + 128, :])
            wc_tiles = []
            for k in range(8):
                w = wc_pool.tile([128, G4], w8, name=f"wc{k}", tag=f"wc{k}")
                nc.sync.dma_start(w, wc_d[128 * k:128 * k + 128, :])
                wc_tiles.append(w)

            c_sb = c_pool.tile([128, 256], f32, name="c_sb")

            def tail(apsrc, bpsrc, first, scale):
                """Transposed-space gate elementwise. apsrc/bpsrc are the
                A=[i.T|f.T] and B=[g.T|o.T] [128,512] gate images (psum or
                sbuf). Returns (h8, hb): fp8 h.T for the recurrence and
                bf16 h.T for the projection; chunk k at cols 32k."""
                IF = ew_pool.tile([128, 512], f32, name="IF", tag="IF")
                nc.scalar.activation(IF, apsrc, SIG, scale=scale)
                fc = ew_pool.tile([128, 256], f32, name="fc", tag="fc")
                if not first:
                    nc.vector.tensor_mul(fc, IF[:, 256:512], c_sb)
                G = ew_pool.tile([128, 256], f32, name="G", tag="G")
                nc.scalar.activation(G, bpsrc[:, 0:256], TANH, scale=scale)
                O = ew_pool.tile([128, 256], f32, name="O", tag="O")
                nc.scalar.activation(O, bpsrc[:, 256:512], SIG, scale=scale)
                nc.vector.tensor_mul(G, G, IF[:, 0:256])  # i*tanh(g)
                if first:
                    nc.vector.tensor_copy(c_sb, G)        # c = ig (c0 = 0)
                else:
                    nc.vector.tensor_add(c_sb, G, fc)     # c = ig + fc
                nc.scalar.activation(fc, c_sb, TANH)      # tanh(c)
                h8 = state_pool.tile([128, 256], w8, name="h8", tag="h8")
                nc.vector.tensor_mul(h8, O, fc)           # h -> fp8
                hb = state_pool.tile([128, 256], bf16, name="hb", tag="hb")
                nc.vector.tensor_mul(hb, O, fc)           # h -> bf16
                return h8, hb

            def proj(hb):
                pp = pps_pool.tile([DOUT, BLOC], f32, name="pp", tag="pp")
                for k in range(8):
                    nc.tensor.matmul(pp, wo_sb[:, DOUT * k:DOUT * (k + 1)],
                                     hb[:, 32 * k:32 * k + 32],
                                     start=(k == 0), stop=(k == 7))
                return pp

            def proj_out(pp, t):
                osb = ew_pool.tile([DOUT, BLOC], f32, name="osb", tag="osb")
                nc.scalar.activation(osb, pp, IDENT, bias=bo_sb)
                nc.sync.dma_start(out_d[:, t, :], osb)

            # ---- step 0: tail only, from host-precomputed gates ----
            cur8, curb = tail(g0a_sb, g0b_sb, first=True, scale=1.0)
            if KDBG:
                hd = ew_pool.tile([128, 256], f32, name="hd", tag="hd")
                nc.vector.tensor_copy(hd, curb)
                nc.sync.dma_start(hdbg_d[0, :, :], hd)

            # ---- steps 1..T-1 ----
            for t in list(range(1, T)) * _REPS:
                ap = gps_pool.tile([128, 512], f32, name="Aps", tag="Aps")
                bp = gps_pool.tile([128, 512], f32, name="Bps", tag="Bps")
                # NOTE: start=True clears the PSUM bank's has_written bits
                # BANK-wide, so it may appear only on the first matmul into
                # each bank; later first-writes overwrite via unset bits.
                for k in range(8):
                    mv = cur8[:, 32 * k:32 * k + 32]
                    for j in range(16):
                        nc.tensor.matmul(
                            ap[:, 32 * j:32 * j + 32],
                            wc_tiles[k][:, 128 * j:128 * j + 128], mv,
                            start=(k == 0 and j == 0),
                            stop=(k == 7 and j == 15),
                            skip_group_check=True)
                for k in range(8):
                    mv = cur8[:, 32 * k:32 * k + 32]
                    for j in range(16, 32):
                        nc.tensor.matmul(
                            bp[:, 32 * (j - 16):32 * (j - 16) + 32],
                            wc_tiles[k][:, 128 * j:128 * j + 128], mv,
                            start=(k == 0 and j == 16),
                            stop=(k == 7 and j == 31),
                            skip_group_check=True)
                pp = None
                if "no_proj" not in _FLAGS:
                    pp = proj(curb)  # h_t -> out row t-1
                nc.vector.tensor_add(ap, ap, ba_sb)
                nc.vector.tensor_add(bp, bp, bb_sb)
                if KDBG:
                    gd = ew_pool.tile([128, 1024], f32, name="gd", tag="gd")
                    nc.vector.tensor_copy(gd[:, 0:512], ap)
                    nc.vector.tensor_copy(gd[:, 512:1024], bp)
                    nc.sync.dma_start(gdbg_d[t, :, :], gd)
                if "no_tail" not in _FLAGS:
                    cur8, curb = tail(ap, bp, first=False, scale=SW)
                    if KDBG:
                        hd = ew_pool.tile([128, 256], f32, name="hd",
                                          tag="hd")
                        nc.vector.tensor_copy(hd, curb)
                        nc.sync.dma_start(hdbg_d[t, :, :], hd)
                if pp is not None:
                    proj_out(pp, t - 1)
            # ---- final projection of h_T ----
            if "no_proj" not in _FLAGS:
                pp = proj(curb)
                proj_out(pp, T - 1)
    nc.finalize()
    return nc


def _build_sw(T, sw):
    """Build with the fp8 dequant scale baked in (ACT scale immediate)."""
    os.environ["KSW"] = repr(sw)
    try:
        return _build_program(T)
    finally:
        del os.environ["KSW"]


def _np_wdt():
    import ml_dtypes
    return (ml_dtypes.float8_e4m3fn if "wfp8" in _FLAGS
            else ml_dtypes.bfloat16)


def _bias_image(bhalf, sw):
    """[2048] bias -> [128, 512] transposed-space image, pre-divided by
    the dequant scale (ACT computes func(scale*(mm + bias/scale)))."""
    img = (bhalf / sw).astype(np.float32).reshape(16, 128)  # (block j, p)
    img = np.repeat(img.T[:, :, None], BLOC, axis=2)  # (p, j, b)
    return np.ascontiguousarray(img.reshape(128, 512))


def _g0_image(ghalf):
    """g0 [32, 2048] -> transposed-space [128, 512] image."""
    arr = ghalf.T.reshape(16, 128, BLOC)  # (block j, p, b)
    return np.ascontiguousarray(arr.transpose(1, 0, 2).reshape(128, 512))


def _prep(x, W_ih, W_hh, b_ih, b_hh, W_out, b_out):
    """Host-side prep shared by all cores. Returns (common_map, g0, sw)."""
    import ml_dtypes
    wdt = _np_wdt()
    Wc = W_ih + W_hh
    if "wfp8" in _FLAGS:
        sw = float(np.abs(Wc).max()) / 448.0
    else:
        sw = 1.0
    WcT8 = np.ascontiguousarray((Wc / sw).T.astype(wdt))
    WoTb = np.ascontiguousarray(W_out.T.astype(ml_dtypes.bfloat16))
    b = b_ih + b_hh
    g0 = x @ W_ih.T + b  # host: 1/128 of the FLOPs
    common = {
        "WcT8": WcT8, "WoTb": WoTb,
        "biasA": _bias_image(b[0:2048], sw),
        "biasB": _bias_image(b[2048:4096], sw),
        "boC": np.ascontiguousarray(b_out.reshape(DOUT, 1)),
    }
    return common, g0, sw


def _in_maps(x, W_ih, W_hh, b_ih, b_hh, W_out, b_out):
    common, g0, sw = _prep(x, W_ih, W_hh, b_ih, b_hh, W_out, b_out)
    in_maps = []
    for c in range(NCORES):
        g0c = g0[BLOC * c:BLOC * (c + 1)]
        m = dict(common)
        m["g0A"] = _g0_image(g0c[:, 0:2048])
        m["g0B"] = _g0_image(g0c[:, 2048:4096])
        in_maps.append(m)
    return in_maps, sw


def _get_program(T, sw):
    key = (T, sw, frozenset(_FLAGS), _REPS)
    if key not in _CACHE:
        _CACHE[key] = _build_sw(T, sw)
    return _CACHE[key]


def _unshard(results):
    outs = []
    for r in results:
        outs.append(np.ascontiguousarray(r["outT"].transpose(2, 1, 0)))
    return np.concatenate(outs, axis=0)


def kernel(x, W_ih, W_hh, b_ih, b_hh, W_out, b_out, T):
    T = int(T)
    x = np.asarray(x, dtype=np.float32)
    W_ih = np.asarray(W_ih, dtype=np.float32)
    W_hh = np.asarray(W_hh, dtype=np.float32)
    b_ih = np.asarray(b_ih, dtype=np.float32)
    b_hh = np.asarray(b_hh, dtype=np.float32)
    W_out = np.asarray(W_out, dtype=np.float32)
    b_out = np.asarray(b_out, dtype=np.float32)

    from concourse.bass_utils import run_bass_kernel_spmd

    in_maps, sw = _in_maps(x, W_ih, W_hh, b_ih, b_hh, W_out, b_out)
    nc = _get_program(T, sw)
    res = run_bass_kernel_spmd(nc, in_maps, core_ids=list(range(NCORES)))
    kernel.last_results = res.results
    return _unshard(res.results)


# revision 17
# speedup vs baseline: 1.3666x; 1.3666x over previous
"""Trainium2 Bass kernel for an autoregressive LSTM decompressor.

Reference math:
  step 0:    gates = x @ W_ih.T + b            (h = c = 0)
  step t>=1: gates = h_{t-1} @ (W_ih+W_hh).T + b    (input == previous hidden)
  i,f,g,o = split(gates); c = sig(f)*c + sig(i)*tanh(g); h = sig(o)*tanh(c)
  out[b,t,:] = h_{t+1} @ W_out.T + b_out  (rows are h_1..h_T)

Sharding: data-parallel, batch 256 -> 32 per core across 8 cores; combined
weights (W_ih+W_hh) resident in SBUF.

Layout (the whole point of this kernel): per-core batch is only 32, so a
batch-stationary matmul would use 32 of the PE array's 128 columns and
stream 32K weight columns per step. Instead the matmuls run
WEIGHT-STATIONARY in transposed space: stationary = Wc.T block
[K=128 h-rows, M=128 gate-rows] (full array), moving = h.T chunk
[128, 32]. bf16 weights make the per-tile LDWEIGHTS use the 2x
fast-weight-load path, which is what bounds this layout (an fp8 variant
behind the "wfp8" flag is ~10% faster but cuts the accuracy margin from
9x to 1.5x, so bf16 ships). Gates come out TRANSPOSED
([gate-rows, batch]), so:
  - gate PSUM banks pack 16 [128,32] blocks: bank A = i|f, bank B = g|o,
    and every elementwise tail op runs at full 128-lane width;
  - the (fp8-mode) global dequant scale rides the ACT activation
    `scale` immediate (bias images are pre-divided by it on the host);
  - h.T is produced directly -- NO transposes anywhere in the loop;
  - the output projection is also weight-stationary ([dout, batch] out),
    with per-partition b_out bias riding the ACT bias port; it runs in
    the PE's tail window. Projection uses a bf16 copy of h.
Step 0's gates (x @ W_ih.T + b, 1/128 of the FLOPs) are precomputed on
the host so W_ih is never shipped; the output is returned transposed per
core and fixed up on the host.
"""

import os
import numpy as np

B, H, DOUT = 256, 1024, 128
NCORES = 8
BLOC = B // NCORES  # 32
G4 = 4 * H  # 4096

_CACHE = {}
_FLAGS = set()  # experiment flags: no_tail, no_proj, wfp8
_REPS = 1  # timing experiments: repeat the steady-state loop


def _wdt():
    import concourse.mybir as mybir
    return mybir.dt.float8e4 if "wfp8" in _FLAGS else mybir.dt.bfloat16


def _build_program(T):
    import concourse.mybir as mybir
    from concourse import bacc
    from concourse.tile import TileContext

    f32 = mybir.dt.float32
    bf16 = mybir.dt.bfloat16
    w8 = _wdt()
    SIG = mybir.ActivationFunctionType.Sigmoid
    TANH = mybir.ActivationFunctionType.Tanh
    IDENT = mybir.ActivationFunctionType.Identity

    nc = bacc.Bacc("TRN2", target_bir_lowering=False, debug=False,
                   num_devices=NCORES)

    wc_d = nc.dram_tensor("WcT8", [H, G4], w8, kind="ExternalInput").ap()
    wo_d = nc.dram_tensor("WoTb", [H, DOUT], bf16, kind="ExternalInput").ap()
    ba_d = nc.dram_tensor("biasA", [128, 512], f32, kind="ExternalInput").ap()
    bb_d = nc.dram_tensor("biasB", [128, 512], f32, kind="ExternalInput").ap()
    bo_d = nc.dram_tensor("boC", [DOUT, 1], f32, kind="ExternalInput").ap()
    g0a_d = nc.dram_tensor("g0A", [128, 512], f32, kind="ExternalInput").ap()
    g0b_d = nc.dram_tensor("g0B", [128, 512], f32, kind="ExternalInput").ap()
    # transposed output; host rearranges to [BLOC, T, DOUT]
    out_d = nc.dram_tensor("outT", [DOUT, T, BLOC], f32,
                           kind="ExternalOutput").ap()
    SW = float(os.environ.get("KSW", "1.0"))  # fp8 dequant scale immediate
    KDBG = bool(os.environ.get("KDBG"))
    if KDBG:
        gdbg_d = nc.dram_tensor("gdbg", [T, 128, 1024], f32,
                                kind="ExternalOutput").ap()
        hdbg_d = nc.dram_tensor("hdbg", [T, 128, 256], f32,
                                kind="ExternalOutput").ap()

    with TileContext(nc) as tc:
        with (
            tc.tile_pool(name="const", bufs=1) as const_pool,
            tc.tile_pool(name="wc", bufs=1) as wc_pool,
            tc.tile_pool(name="state", bufs=2) as state_pool,
            tc.tile_pool(name="cpool", bufs=1) as c_pool,
            tc.tile_pool(name="ew", bufs=2) as ew_pool,
            tc.tile_pool(name="gates_ps", bufs=2, space="PSUM") as gps_pool,
            tc.tile_pool(name="proj_ps", bufs=2, space="PSUM") as pps_pool,
        ):
            ba_sb = const_pool.tile([128, 512], f32, name="ba_sb")
            nc.sync.dma_start(ba_sb, ba_d)
            bb_sb = const_pool.tile([128, 512], f32, name="bb_sb")
            nc.sync.dma_start(bb_sb, bb_d)
            bo_sb = const_pool.tile([DOUT, 1], f32, name="bo_sb")
            nc.sync.dma_start(bo_sb, bo_d)
            g0a_sb = const_pool.tile([128, 512], f32, name="g0a_sb")
            nc.sync.dma_start(g0a_sb, g0a_d)
            g0b_sb = const_pool.tile([128, 512], f32, name="g0b_sb")
            nc.sync.dma_start(g0b_sb, g0b_d)
            wo_sb = const_pool.tile([128, DOUT * 8], bf16, name="wo_sb")
            for k in range(8):
                nc.sync.dma_start(wo_sb[:, DOUT * k:DOUT * (k + 1)],
                                  wo_d[128 * k:128 * k + 128, :])
            wc_tiles = []
            for k in range(8):
                w = wc_pool.tile([128, G4], w8, name=f"wc{k}", tag=f"wc{k}")
                nc.sync.dma_start(w, wc_d[128 * k:128 * k + 128, :])
                wc_tiles.append(w)

            c_sb = c_pool.tile([128, 256], f32, name="c_sb")

            def tail(apsrc, bpsrc, first, scale):
                """Transposed-space gate elementwise. apsrc/bpsrc are the
                A=[i.T|f.T] and B=[g.T|o.T] [128,512] gate images (psum or
                sbuf). Returns (h8, hb): fp8 h.T for the recurrence and
                bf16 h.T for the projection; chunk k at cols 32k."""
                IF = ew_pool.tile([128, 512], f32, name="IF", tag="IF")
                nc.scalar.activation(IF, apsrc, SIG, scale=scale)
                fc = ew_pool.tile([128, 256], f32, name="fc", tag="fc")
                if not first:
                    nc.vector.tensor_mul(fc, IF[:, 256:512], c_sb)
                G = ew_pool.tile([128, 256], f32, name="G", tag="G")
                nc.scalar.activation(G[:, 0:128], bpsrc[:, 0:128], TANH,
                                     scale=scale)
                nc.scalar.activation(G[:, 128:256], bpsrc[:, 128:256], TANH,
                                     scale=scale)
                O = ew_pool.tile([128, 256], f32, name="O", tag="O")
                nc.scalar.activation(O, bpsrc[:, 256:512], SIG, scale=scale)
                h8 = state_pool.tile([128, 256], w8, name="h8", tag="h8")
                hb = state_pool.tile([128, 256], bf16, name="hb", tag="hb")
                # hidden-half split: half hh covers h chunks 4hh..4hh+3, so
                # the next step's first matmuls launch off half 0 while
                # half 1 is still in the ACT/DVE pipe.
                for hh in range(2):
                    sl = slice(128 * hh, 128 * hh + 128)
                    nc.vector.tensor_mul(G[:, sl], G[:, sl],
                                         IF[:, sl])       # i*tanh(g)
                    if first:
                        nc.vector.tensor_copy(c_sb[:, sl], G[:, sl])
                    else:
                        nc.vector.tensor_add(c_sb[:, sl], G[:, sl],
                                             fc[:, sl])   # c = ig + fc
                    nc.scalar.activation(fc[:, sl], c_sb[:, sl], TANH)
                    nc.vector.tensor_mul(h8[:, sl], O[:, sl], fc[:, sl])
                for hh in range(2):
                    sl = slice(128 * hh, 128 * hh + 128)
                    nc.vector.tensor_mul(hb[:, sl], O[:, sl], fc[:, sl])
                return h8, hb

            def proj(hb):
                pp = pps_pool.tile([DOUT, BLOC], f32, name="pp", tag="pp")
                for k in range(8):
                    nc.tensor.matmul(pp, wo_sb[:, DOUT * k:DOUT * (k + 1)],
                                     hb[:, 32 * k:32 * k + 32],
                                     start=(k == 0), stop=(k == 7))
                return pp

            def proj_out(pp, t):
                osb = ew_pool.tile([DOUT, BLOC], f32, name="osb", tag="osb")
                nc.scalar.activation(osb, pp, IDENT, bias=bo_sb)
                nc.sync.dma_start(out_d[:, t, :], osb)

            # ---- step 0: tail only, from host-precomputed gates ----
            cur8, curb = tail(g0a_sb, g0b_sb, first=True, scale=1.0)
            if KDBG:
                hd = ew_pool.tile([128, 256], f32, name="hd", tag="hd")
                nc.vector.tensor_copy(hd, curb)
                nc.sync.dma_start(hdbg_d[0, :, :], hd)

            # ---- steps 1..T-1 ----
            for t in list(range(1, T)) * _REPS:
                ap = gps_pool.tile([128, 512], f32, name="Aps", tag="Aps")
                bp = gps_pool.tile([128, 512], f32, name="Bps", tag="Bps")
                # NOTE: start=True clears the PSUM bank's has_written bits
                # BANK-wide, so it may appear only on the first matmul into
                # each bank; later first-writes overwrite via unset bits.
                for k in range(8):
                    mv = cur8[:, 32 * k:32 * k + 32]
                    for j in range(16):
                        nc.tensor.matmul(
                            ap[:, 32 * j:32 * j + 32],
                            wc_tiles[k][:, 128 * j:128 * j + 128], mv,
                            start=(k == 0 and j == 0),
                            stop=(k == 7 and j == 15),
                            skip_group_check=True)
                for k in range(8):
                    mv = cur8[:, 32 * k:32 * k + 32]
                    for j in range(16, 32):
                        nc.tensor.matmul(
                            bp[:, 32 * (j - 16):32 * (j - 16) + 32],
                            wc_tiles[k][:, 128 * j:128 * j + 128], mv,
                            start=(k == 0 and j == 16),
                            stop=(k == 7 and j == 31),
                            skip_group_check=True)
                pp = None
                if "no_proj" not in _FLAGS:
                    pp = proj(curb)  # h_t -> out row t-1
                nc.vector.tensor_add(ap, ap, ba_sb)
                nc.vector.tensor_add(bp[:, 0:256], bp[:, 0:256],
                                     bb_sb[:, 0:256])
                nc.vector.tensor_add(bp[:, 256:512], bp[:, 256:512],
                                     bb_sb[:, 256:512])
                if KDBG:
                    gd = ew_pool.tile([128, 1024], f32, name="gd", tag="gd")
                    nc.vector.tensor_copy(gd[:, 0:512], ap)
                    nc.vector.tensor_copy(gd[:, 512:1024], bp)
                    nc.sync.dma_start(gdbg_d[t, :, :], gd)
                if "no_tail" not in _FLAGS:
                    cur8, curb = tail(ap, bp, first=False, scale=SW)
                    if KDBG:
                        hd = ew_pool.tile([128, 256], f32, name="hd",
                                          tag="hd")
                        nc.vector.tensor_copy(hd, curb)
                        nc.sync.dma_start(hdbg_d[t, :, :], hd)
                if pp is not None:
                    proj_out(pp, t - 1)
            # ---- final projection of h_T ----
            if "no_proj" not in _FLAGS:
                pp = proj(curb)
                proj_out(pp, T - 1)
    nc.finalize()
    return nc


def _build_sw(T, sw):
    """Build with the fp8 dequant scale baked in (ACT scale immediate)."""
    os.environ["KSW"] = repr(sw)
    try:
        return _build_program(T)
    finally:
        del os.environ["KSW"]


def _np_wdt():
    import ml_dtypes
    return (ml_dtypes.float8_e4m3fn if "wfp8" in _FLAGS
            else ml_dtypes.bfloat16)


def _bias_image(bhalf, sw):
    """[2048] bias -> [128, 512] transposed-space image, pre-divided by
    the dequant scale (ACT computes func(scale*(mm + bias/scale)))."""
    img = (bhalf / sw).astype(np.float32).reshape(16, 128)  # (block j, p)
    img = np.repeat(img.T[:, :, None], BLOC, axis=2)  # (p, j, b)
    return np.ascontiguousarray(img.reshape(128, 512))


def _g0_image(ghalf):
    """g0 [32, 2048] -> transposed-space [128, 512] image."""
    arr = ghalf.T.reshape(16, 128, BLOC)  # (block j, p, b)
    return np.ascontiguousarray(arr.transpose(1, 0, 2).reshape(128, 512))


def _prep(x, W_ih, W_hh, b_ih, b_hh, W_out, b_out):
    """Host-side prep shared by all cores. Returns (common_map, g0, sw)."""
    import ml_dtypes
    wdt = _np_wdt()
    Wc = W_ih + W_hh
    if "wfp8" in _FLAGS:
        # device e4m3 rejects the 448-max encodings; clamp to 240
        sw = float(np.abs(Wc).max()) / 240.0
    else:
        sw = 1.0
    WcT8 = np.ascontiguousarray(np.clip(Wc / sw, -240, 240).T.astype(wdt))
    WoTb = np.ascontiguousarray(W_out.T.astype(ml_dtypes.bfloat16))
    b = b_ih + b_hh
    g0 = x @ W_ih.T + b  # host: 1/128 of the FLOPs
    common = {
        "WcT8": WcT8, "WoTb": WoTb,
        "biasA": _bias_image(b[0:2048], sw),
        "biasB": _bias_image(b[2048:4096], sw),
        "boC": np.ascontiguousarray(b_out.reshape(DOUT, 1)),
    }
    return common, g0, sw


def _in_maps(x, W_ih, W_hh, b_ih, b_hh, W_out, b_out):
    common, g0, sw = _prep(x, W_ih, W_hh, b_ih, b_hh, W_out, b_out)
    in_maps = []
    for c in range(NCORES):
        g0c = g0[BLOC * c:BLOC * (c + 1)]
        m = dict(common)
        m["g0A"] = _g0_image(g0c[:, 0:2048])
        m["g0B"] = _g0_image(g0c[:, 2048:4096])
        in_maps.append(m)
    return in_maps, sw


def _get_program(T, sw):
    key = (T, sw, frozenset(_FLAGS), _REPS)
    if key not in _CACHE:
        _CACHE[key] = _build_sw(T, sw)
    return _CACHE[key]


def _unshard(results):
    outs = []
    for r in results:
        outs.append(np.ascontiguousarray(r["outT"].transpose(2, 1, 0)))
    return np.concatenate(outs, axis=0)


def kernel(x, W_ih, W_hh, b_ih, b_hh, W_out, b_out, T):
    T = int(T)
    x = np.asarray(x, dtype=np.float32)
    W_ih = np.asarray(W_ih, dtype=np.float32)
    W_hh = np.asarray(W_hh, dtype=np.float32)
    b_ih = np.asarray(b_ih, dtype=np.float32)
    b_hh = np.asarray(b_hh, dtype=np.float32)
    W_out = np.asarray(W_out, dtype=np.float32)
    b_out = np.asarray(b_out, dtype=np.float32)

    from concourse.bass_utils import run_bass_kernel_spmd

    in_maps, sw = _in_maps(x, W_ih, W_hh, b_ih, b_hh, W_out, b_out)
    nc = _get_program(T, sw)
    res = run_bass_kernel_spmd(nc, in_maps, core_ids=list(range(NCORES)))
    kernel.last_results = res.results
    return _unshard(res.results)
